# revision 20
# baseline (speedup 1.0000x reference)
"""Cross-attention Trainium2 Bass kernel.

Full inputs in, full output out. Internally: 8-way sharding, data-parallel
over batch (B=2) x tensor-parallel over head groups (16 heads -> 4 groups
of 4). Core c handles batch c//4 and head-group c%4 (see _in_maps). Each
core computes a partial output (its 4 heads' contribution through W_o);
the host sums the 4 partials per batch and adds b_o.

Device-side layout notes:
  - All matmuls run in bf16 (fp32 accumulate in PSUM).
  - Activations are fed pre-transposed (D on partitions) so projection
    matmuls need no on-device transpose.
  - Attention computes S^T tiles [k x q]; exp on the Act engine reads a
    two-bank [128, 1024] PSUM tile (both heads of an mc pair at once).
  - A@V runs in Y^T form: lhsT = V chunk augmented with a ones column
    (so PSUM row 64 accumulates the softmax denominators), rhs = the
    exp'd S^T tiles streamed 512 columns at a time. No PE transposes.
  - Softmax normalization: DVE reciprocal of the sums row, gpsimd
    partition-broadcast across the 64 head-dim rows, DVE multiply.
  - PE instruction order interleaves projection / O-projection matmuls
    between S^T matmuls so the tensor engine stream stays dense (keeps
    the HAM clock gate at K=8/8), with A@V lagging one q-group behind.
"""

import numpy as np
import ml_dtypes

T = 2048          # T_dec == T_enc
D = 1024          # d_model
P = 128
HPC = 4           # heads per core
DH = 64           # head dim
KC = D // P       # 8 contraction chunks for projections
NKT = T // P      # 16 key chunks
QG = 512          # q-group width
NQG = T // QG     # 4 q groups
NCORES = 8

_PROGRAM = None


def _split_multiwaits_json(raw: bytes) -> bytes:
    """This walrus build accepts at most ONE sync-wait per instruction.
    Split every multi-wait instruction into single-wait same-engine NoOps
    followed by the instruction (same-engine program order preserves
    semantics exactly)."""
    try:
        import orjson as _json

        loads, dumps = _json.loads, _json.dumps
    except ImportError:
        import json as _json

        loads = _json.loads
        dumps = lambda o: _json.dumps(o).encode()

    j = loads(raw)
    k = 0
    for fn in j["functions"]:
        for bb in fn["blocks"]:
            insts = bb["instructions"]
            out = []
            changed = False
            for inst in insts:
                si = inst.get("sync_info")
                waits = (si.get("on_wait") or []) if si else []
                if len(waits) > 1:
                    for w in waits[:-1]:
                        nop = {
                            "engine": inst["engine"],
                            "ins": [],
                            "outs": [],
                            "name": f"{inst['name']}-sw{k}",
                            "opcode": "NoOp",
                            "sync_info": {"on_update": [], "on_wait": [w]},
                        }
                        if inst.get("debug") is not None:
                            nop["debug"] = inst["debug"]
                        out.append(nop)
                        k += 1
                    si["on_wait"] = [waits[-1]]
                    changed = True
                out.append(inst)
            if changed:
                bb["instructions"] = out
    return dumps(j)


def _build_program():
    import concourse.bass as bass
    import concourse.tile as tile
    import concourse.mybir as mybir
    from concourse.vector_clock import ScopedClock
    from contextlib import ExitStack

    f32 = mybir.dt.float32
    bf16 = mybir.dt.bfloat16
    EXP = mybir.ActivationFunctionType.Exp

    class _TC(tile.TileContext):
        # This walrus build rejects >1 sync waits on the CTRL Drain
        # encoding; split the kernel-tail drain's waits into single-wait
        # SP instructions instead.
        def _drain_and_barrier(self, tick_clock, wait_clock):
            dummy = mybir.InstNoOp(
                name="wait-collector", engine=mybir.EngineType.SP
            )
            wait_clock.add_sem_waits(
                dummy, ScopedClock({None: tick_clock.global_clock})
            )
            si = dummy.sync_info
            waits = list(si.on_wait) if si and si.on_wait else []
            assert self.sems is not None
            by_name = {h.name: h for h in self.sems.allocated().values()}
            for w in waits:
                self.nc.sync.wait_ge(by_name[w.ant_name], w.wait_value)
            self.nc.sync.drain()
            self.nc.all_engine_barrier()
            popped = self.nc._tile_sem_poison_stack.pop()
            assert popped is self._sem_poison
            self.nc.clear_and_free_semaphores(
                list(self.sems.allocated().values())
            )
            self.nc.all_engine_barrier()

    nc = bass.Bass()

    tgtT = nc.dram_tensor("tgtT", [D, T], bf16, kind="ExternalInput")
    memT = nc.dram_tensor("memT", [D, T], bf16, kind="ExternalInput")
    wq = nc.dram_tensor("wq", [D, 256], bf16, kind="ExternalInput")
    wk = nc.dram_tensor("wk", [D, 256], bf16, kind="ExternalInput")
    wv = nc.dram_tensor("wv", [D, 256], bf16, kind="ExternalInput")
    wo = nc.dram_tensor("wo", [256, D], bf16, kind="ExternalInput")
    bq = nc.dram_tensor("bq", [256], f32, kind="ExternalInput")
    bk = nc.dram_tensor("bk", [256], f32, kind="ExternalInput")
    bv = nc.dram_tensor("bv", [256], f32, kind="ExternalInput")
    out = nc.dram_tensor("out", [T, D], f32, kind="ExternalOutput")

    with _TC(nc) as tc, ExitStack() as ctx:
        singles = ctx.enter_context(tc.tile_pool(name="singles", bufs=1))

        # ---- weights + biases ----
        wq_s = singles.tile([P, KC, 256], bf16, tag="wq")
        wk_s = singles.tile([P, KC, 256], bf16, tag="wk")
        wv_s = singles.tile([P, KC, 256], bf16, tag="wv")
        wo_s = singles.tile([P, 2, D], bf16, tag="wo")
        nc.sync.dma_start(wq_s, wq.rearrange("(c p) n -> p c n", p=P))
        nc.sync.dma_start(wk_s, wk.rearrange("(c p) n -> p c n", p=P))
        nc.sync.dma_start(wv_s, wv.rearrange("(c p) n -> p c n", p=P))
        nc.sync.dma_start(wo_s, wo.rearrange("(c p) n -> p c n", p=P))

        bq_s = singles.tile([P, 2], f32, tag="bq")
        bk_s = singles.tile([P, 2], f32, tag="bk")
        bvb = singles.tile([P, 256], f32, tag="bvb")
        nc.sync.dma_start(bq_s, bq.rearrange("(c p) -> p c", p=P))
        nc.sync.dma_start(bk_s, bk.rearrange("(c p) -> p c", p=P))
        bv_ap = bass.AP(tensor=bv[:].tensor, offset=0, ap=[[0, P], [1, 256]])
        nc.sync.dma_start(bvb, bv_ap)

        # ---- transposed activations, chunked so PE can start early ----
        mT = singles.tile([P, KC, T], bf16, tag="mT")
        tT = singles.tile([P, KC, T], bf16, tag="tT")
        memT_r = memT.rearrange("(c p) t -> p c t", p=P)
        tgtT_r = tgtT.rearrange("(c p) t -> p c t", p=P)
        nc.sync.dma_start(mT[:, :, 0:QG], memT_r[:, :, 0:QG])
        nc.sync.dma_start(tT[:, :, 0:QG], tgtT_r[:, :, 0:QG])
        for g in range(1, NQG):
            sl = slice(g * QG, (g + 1) * QG)
            nc.sync.dma_start(mT[:, :, sl], memT_r[:, :, sl])
        for g in range(1, NQG):
            sl = slice(g * QG, (g + 1) * QG)
            nc.sync.dma_start(tT[:, :, sl], tgtT_r[:, :, sl])

        # persistent intermediates
        qT = singles.tile([P, 2, T], bf16, tag="qT")   # Q^T (heads on rows)
        kT = singles.tile([P, 2, T], bf16, tag="kT")   # K^T
        # V' per key-chunk: [V | 1] for every head (ones col -> softmax sums)
        vS = singles.tile([P, NKT, HPC, DH + 2], bf16, tag="vS")
        yT = singles.tile([P, 2, T], bf16, tag="yT")   # normalized Y^T

        nc.vector.memset(vS[:, :, :, DH : DH + 1], 1.0)

        # ones row on lane 64 — stationary operand of the K=1 broadcast
        # matmuls that replicate each reciprocal-sums row across the 64
        # head-dim partitions.
        ones64 = singles.tile([DH + 1, DH], bf16, tag="ones64")
        nc.vector.memset(ones64[DH : DH + 1, :], 1.0)

        psum = ctx.enter_context(
            tc.tile_pool(name="psum", bufs=2, space="PSUM")
        )
        ptp = ctx.enter_context(tc.tile_pool(name="ptp", bufs=2))
        ysp = ctx.enter_context(tc.tile_pool(name="ysp", bufs=2))
        ogp = ctx.enter_context(tc.tile_pool(name="ogp", bufs=3))

        # ---------- emission helpers ----------
        def emit_kproj(mc, g):
            cols = slice(g * QG, (g + 1) * QG)
            pk = psum.tile([P, QG], f32, tag="pj")
            for c in range(KC):
                nc.tensor.matmul(
                    pk,
                    wk_s[:, c, mc * P : (mc + 1) * P],
                    mT[:, c, cols],
                    start=(c == 0),
                    stop=(c == KC - 1),
                )
            nc.vector.tensor_scalar_add(
                kT[:, mc, cols], pk, bk_s[:, mc : mc + 1]
            )

        def emit_qproj(mc, g):
            cols = slice(g * QG, (g + 1) * QG)
            pq = psum.tile([P, QG], f32, tag="pj")
            for c in range(KC):
                nc.tensor.matmul(
                    pq,
                    wq_s[:, c, mc * P : (mc + 1) * P],
                    tT[:, c, cols],
                    start=(c == 0),
                    stop=(c == KC - 1),
                )
            nc.vector.tensor_scalar_add(
                qT[:, mc, cols], pq, bq_s[:, mc : mc + 1]
            )

        bvb4 = bvb.rearrange("p (h d) -> p h d", h=HPC)

        def emit_vproj(tt):
            pv = psum.tile([P, QG], f32, tag="pj")
            for c in range(KC):
                nc.tensor.matmul(
                    pv[:, 0:256],
                    mT[:, c, tt * P : (tt + 1) * P],
                    wv_s[:, c, :],
                    start=(c == 0),
                    stop=(c == KC - 1),
                )
            pv4 = pv[:, 0:256].rearrange("p (h d) -> p h d", h=HPC)
            nc.vector.tensor_add(vS[:, tt, :, 0:DH], pv4, bvb4)

        def emit_oproj(tt, ng):
            qrows = slice(tt * P, (tt + 1) * P)
            ncols = slice(ng * QG, (ng + 1) * QG)
            po = psum.tile([P, QG], f32, tag="pj")
            for mc in range(2):
                nc.tensor.matmul(
                    po,
                    yT[:, mc, qrows],
                    wo_s[:, mc, ncols],
                    start=(mc == 0),
                    stop=(mc == 1),
                )
            og = ogp.tile([P, QG], f32, tag="og")
            nc.vector.tensor_copy(og, po)
            nc.sync.dma_start(out[qrows, ncols], og)

        def _spread(fillers):
            """fillers: either a flat list (spread evenly over the 16
            chunk slots) or a dict {slot: [thunks]} for precise
            placement."""
            if isinstance(fillers, dict):
                return [fillers.get(kc, []) for kc in range(NKT)]
            nf = len(fillers)
            return [
                fillers[kc * nf // NKT : (kc + 1) * nf // NKT]
                for kc in range(NKT)
            ]

        def emit_s_group(mc, g, fillers):
            """S^T for all 16 key chunks of (mc, g); exp to a fresh pt
            tile; `fillers` interleaved between chunks to keep the PE
            stream dense while Act chews on the exps."""
            cols = slice(g * QG, (g + 1) * QG)
            pt = ptp.tile([P, NKT, 2 * QG], bf16, tag="pt")
            slots = _spread(fillers)
            for kc in range(NKT):
                krows = slice(kc * P, (kc + 1) * P)
                ps2 = psum.tile([P, 2 * QG], f32, tag="ps2")
                nc.tensor.matmul(
                    ps2[:, 0:QG],
                    kT[0:64, mc, krows],
                    qT[0:64, mc, cols],
                    start=True,
                    stop=True,
                    tile_position=(0, 0),
                )
                nc.tensor.matmul(
                    ps2[:, QG : 2 * QG],
                    kT[64:128, mc, krows],
                    qT[64:128, mc, cols],
                    start=True,
                    stop=True,
                    tile_position=(64, 0),
                )
                nc.scalar.activation(pt[:, kc, :], ps2, EXP, scale=0.125)
                for f in slots[kc]:
                    f()
            return pt

        def emit_av_group(mc, g, pt, fillers=()):
            """A@V in Y^T form for both heads of (mc, g) — h0/h1
            interleaved per key chunk so the two accumulation chains
            pipeline in the PE — plus softmax normalization into yT."""
            cols = slice(g * QG, (g + 1) * QG)
            slots = _spread(
                fillers if isinstance(fillers, dict) else list(fillers)
            )
            py0 = psum.tile([DH + 1, QG], f32, tag="py")
            py1 = psum.tile([DH + 1, QG], f32, tag="py")
            pys = (py0, py1)
            for kc in range(NKT):
                for h in range(2):
                    nc.tensor.matmul(
                        pys[h],
                        vS[:, kc, 2 * mc + h, 0 : DH + 1],
                        pt[:, kc, h * QG : (h + 1) * QG],
                        start=(kc == 0),
                        stop=(kc == NKT - 1),
                    )
                for f in slots[kc]:
                    f()
            # Reciprocal of the sums rows now (DVE, ~3.4us each, runs
            # while the PE moves on); the PE broadcast + normalize is
            # returned as thunks the caller schedules a few matmuls
            # later, by which point the reciprocal is done.
            rss = []
            for h in range(2):
                rs = ysp.tile([DH + 1, QG], bf16, tag="rs")
                with nc.allow_low_precision(
                    reason="bf16 softmax denominators for PE broadcast"
                ):
                    nc.vector.reciprocal(
                        rs[DH : DH + 1, :], pys[h][DH : DH + 1, :]
                    )
                rss.append(rs)

            def norm(h):
                py, rs = pys[h], rss[h]
                # broadcast recip down the 64 head-dim rows (K=1 matmul)
                bc = psum.tile([P, QG], f32, tag="pj")
                nc.tensor.matmul(
                    bc[0:DH, :],
                    ones64[DH : DH + 1, :],
                    rs[DH : DH + 1, :],
                    start=True,
                    stop=True,
                )
                # DVE can read only one PSUM operand per op — stage the
                # broadcast rows in SBUF before the normalize multiply.
                bcs = ysp.tile([DH, QG], bf16, tag="bcs")
                nc.vector.tensor_copy(bcs, bc[0:DH, :])
                if h == 0:
                    nc.vector.tensor_mul(
                        yT[0:DH, mc, cols], py[0:DH, :], bcs
                    )
                else:
                    ytmp = ysp.tile([DH, QG], bf16, tag="ytmp")
                    nc.vector.tensor_mul(ytmp, py[0:DH, :], bcs)
                    nc.sync.dma_start(yT[DH:P, mc, cols], ytmp)

            return [lambda: norm(0), lambda: norm(1)]

        # ---------- emission schedule ----------
        # Minimal prefix so S(0,0) (and with it the Act engine) starts
        # ASAP; all remaining projections are fillers between S matmuls.
        def K(mc, g):
            return lambda: emit_kproj(mc, g)

        def Q(mc, g):
            return lambda: emit_qproj(mc, g)

        def V(tt):
            return lambda: emit_vproj(tt)

        def O(tt, ng):
            return lambda: emit_oproj(tt, ng)

        def o_slots(tts, base):
            d = {}
            pairs = [(tt, ng) for tt in tts for ng in range(2)]
            span = NKT - base
            for i, (tt, ng) in enumerate(pairs):
                slot = base + (i * span) // len(pairs)
                assert slot < NKT
                d.setdefault(slot, []).append(O(tt, ng))
            return d

        emit_kproj(0, 0)
        emit_qproj(0, 0)

        def merge(*dicts):
            out = {}
            for dd in dicts:
                for k, v in dd.items():
                    out.setdefault(k, []).extend(v)
            return out

        group_fillers = {
            (0, 0): {
                1: [K(0, 1)],
                4: [K(0, 2)],
                7: [K(0, 3)],
                9: [Q(0, 1)],
                11: [V(0)],
                12: [V(1)],
                13: [V(2)],
                14: [V(3)],
                15: [V(4)],
            },
            (0, 1): [V(t) for t in range(5, 16)] + [Q(0, 2)],
            (0, 2): {4: [Q(0, 3)], 8: [K(1, 0)], 12: [K(1, 1)]},
            (0, 3): {4: [K(1, 2)], 8: [K(1, 3)], 12: [Q(1, 0)]},
            (1, 0): {5: [Q(1, 1)], 10: [Q(1, 2)]},
            (1, 1): {8: [Q(1, 3)]},
            # O-projection fillers must come AFTER this group's norm
            # thunks (slots 5/9) — the q-group's mc=1 yT halves are
            # written by the norm that lands in this same S group.
            (1, 2): o_slots(range(0, 4), 10),
            (1, 3): o_slots(range(4, 8), 10),
        }

        prev = None   # (mc, g, pt) awaiting A@V
        norms = []    # pending normalization thunks from the last A@V
        for mc in range(2):
            for g in range(NQG):
                fillers = group_fillers[(mc, g)]
                if norms:
                    if not isinstance(fillers, dict):
                        fillers = _spread(fillers)
                        fillers = {i: s for i, s in enumerate(fillers)}
                    fillers = merge({5: [norms[0]], 9: [norms[1]]}, fillers)
                    norms = []
                pt = emit_s_group(mc, g, fillers)
                if prev is not None:
                    norms = emit_av_group(prev[0], prev[1], prev[2])
                prev = (mc, g, pt)
        # tail: norm of the second-to-last group BEFORE the last A@V is
        # emitted (its py buffers get recycled by that A@V), then the
        # last A@V with O(g2) as fillers, then the final norm + O(g3).
        norms[0]()
        norms[1]()
        norms = emit_av_group(
            prev[0], prev[1], prev[2], o_slots(range(8, 12), 4)
        )
        norms[0]()
        norms[1]()
        for tt in range(12, 16):
            for ng in range(2):
                emit_oproj(tt, ng)

    _orig_to_json = nc.to_json_bytes
    nc.to_json_bytes = lambda: _split_multiwaits_json(_orig_to_json())
    return nc


def _get_program():
    global _PROGRAM
    if _PROGRAM is None:
        _PROGRAM = _build_program()
    return _PROGRAM


def _in_maps(tgt, memory, W_q, b_q, W_k, b_k, W_v, b_v, W_o):
    bf16 = ml_dtypes.bfloat16
    maps = []
    tT = [np.ascontiguousarray(tgt[b].T).astype(bf16) for b in range(2)]
    mT = [np.ascontiguousarray(memory[b].T).astype(bf16) for b in range(2)]
    for c in range(NCORES):
        b, hg = c // HPC, c % HPC
        sl = slice(hg * 256, (hg + 1) * 256)
        maps.append(
            {
                "tgtT": tT[b],
                "memT": mT[b],
                "wq": np.ascontiguousarray(W_q[:, sl]).astype(bf16),
                "wk": np.ascontiguousarray(W_k[:, sl]).astype(bf16),
                "wv": np.ascontiguousarray(W_v[:, sl]).astype(bf16),
                "wo": np.ascontiguousarray(W_o[sl, :]).astype(bf16),
                "bq": np.ascontiguousarray(b_q[sl]).astype(np.float32),
                "bk": np.ascontiguousarray(b_k[sl]).astype(np.float32),
                "bv": np.ascontiguousarray(b_v[sl]).astype(np.float32),
            }
        )
    return maps


def kernel(tgt, memory, W_q, b_q, W_k, b_k, W_v, b_v, W_o, b_o):
    from concourse.bass_utils import run_bass_kernel_spmd

    tgt = np.asarray(tgt)
    memory = np.asarray(memory)
    nc = _get_program()
    maps = _in_maps(
        np.asarray(tgt, np.float32),
        np.asarray(memory, np.float32),
        np.asarray(W_q, np.float32),
        np.asarray(b_q, np.float32),
        np.asarray(W_k, np.float32),
        np.asarray(b_k, np.float32),
        np.asarray(W_v, np.float32),
        np.asarray(b_v, np.float32),
        np.asarray(W_o, np.float32),
    )
    res = run_bass_kernel_spmd(nc, maps, core_ids=list(range(NCORES)))
    outs = [r["out"] for r in res.results]
    bo = np.asarray(b_o, np.float64)
    full = np.empty((2, T, D), np.float32)
    for b in range(2):
        acc = np.zeros((T, D), np.float64)
        for hg in range(HPC):
            acc += outs[b * HPC + hg].astype(np.float64)
        full[b] = (acc + bo).astype(np.float32)
    return full
